# revision 27
# baseline (speedup 1.0000x reference)
"""Multi-headed self-attention on 8 Trainium2 NeuronCores (Bass/Tile).

Problem: B=8, S=1024, D=1024, H=16 heads (DH=64), fp32.
    qp = q @ Wq.T + bq ; kp = k @ Wk.T + bk ; vp = v @ Wv.T + bv
    out = softmax(Qh Kh^T / sqrt(DH) + maskbias) Vh   (per head, merged)

Sharding: data-parallel over batch - one batch element per core. The
host pre-transposes inputs/weights (layout only; all FLOPs on device),
casts the big operands to bf16, and folds the 1/sqrt(DH) softmax scale
into Wq/bq (exact rescale, done in fp32 before the bf16 cast).

Per-core device algorithm (all matmuls bf16, PSUM accumulation fp32):
  A. v-projection first into v_aug [S, H*(DH+1)] = per head
     [V columns | ones column] (the ones column makes the AV matmul
     produce the softmax denominator for free in its last row).
  B. Software-pipelined main loop over 16 groups (= 8 head-pairs x 2
     q-chunks of 512). Per group, per k-tile (8 of 128 positions):
       - scores: TWO row-tiled K=64 matmuls (head A on PE row-group 0,
         head B on row-group 64 - they execute concurrently on the
         tiled PE array) write the two halves of one [128,1024] PSUM
         tile: scoresT[k, qA | qB].
       - exp on the Scalar engine: one ACT instruction per k-tile
         covers both heads ([128,1024] fp32 PSUM -> bf16 SBUF), with
         the additive mask bias as a per-partition bias AP (zeros for
         an all-ones mask; same compiled program handles any mask).
     Interleaved into the same PE instruction stream as filler so the
     PE never idles while ACT chews on exp:
       - AV matmuls of the PREVIOUS group ([vh|1].T @ expT,
         accumulating [65, qA | qB] over k-tiles),
       - projection matmuls of the NEXT head-pair's qp/kp tiles,
       - output transposes of the group before that (regular bf16
         matmul against an identity), followed by DVE reciprocal of
         the denominator column and a per-partition tensor_scalar
         multiply that writes normalized [q, d] output tiles, streamed
         to DRAM as [128,128] blocks.

Engine budget per core (approx, warm): PE ~185us (the bottleneck:
projections 83 + scores 28 row-tiled + AV 55 + transposes/LDW),
Scalar/ACT exp ~132us, DVE ~85us, all overlapped under PE.
"""

import os
import sys

for _p in (
    "/root/.axon_site",
    "/root/.axon_site/_ro/trn_rl_repo",
    "/root/.axon_site/_ro/pypackages",
    "/opt/trn_rl_repo",
):
    if os.path.isdir(_p) and _p not in sys.path:
        sys.path.append(_p)

import numpy as np
import ml_dtypes

import concourse.bass as bass
import concourse.tile as tile
import concourse.mybir as mybir
from concourse import bacc
from concourse.bass_utils import run_bass_kernel_spmd
from concourse.masks import make_identity

B, S, D, H = 8, 1024, 1024, 16
DH = D // H  # 64
N_CORES = 8
P = 128  # partitions

F32 = mybir.dt.float32
BF16 = mybir.dt.bfloat16
BF16NP = ml_dtypes.bfloat16


def build_bass(s=S, d=D, h=H, debug=False):
    """Build the per-core Bass program (same program on all 8 cores)."""
    dh = d // h
    kt_n = d // P          # contraction tiles for projections (8)
    ot_n = d // P          # output-feature tiles = head pairs (8)
    st_n = s // P          # sequence tiles of 128 (8)
    ch = 512               # q-chunk (PSUM fp32 bank = 512 cols)
    ch_n = s // ch         # q-chunks (2)
    qb_n = ch // P         # 128-row q blocks per chunk (4)
    hp_n = P // dh         # heads per pair tile (2)
    vaug_w = h * (dh + 1)  # v_aug width (1040)

    nc = bacc.Bacc(
        "TRN2", target_bir_lowering=False, debug=debug, num_devices=N_CORES
    )

    qT = nc.dram_tensor("qT", (d, s), BF16, kind="ExternalInput").ap()
    kT = nc.dram_tensor("kT", (d, s), BF16, kind="ExternalInput").ap()
    vT = nc.dram_tensor("vT", (d, s), BF16, kind="ExternalInput").ap()
    wqT = nc.dram_tensor("wqT", (d, d), BF16, kind="ExternalInput").ap()
    wkT = nc.dram_tensor("wkT", (d, d), BF16, kind="ExternalInput").ap()
    wvT = nc.dram_tensor("wvT", (d, d), BF16, kind="ExternalInput").ap()
    # packed constants: [bqT (8) | bkT (8) | mb (8) | bvB (1040)] - one
    # wide DMA instead of four tiny (32B-line) ones
    consts = nc.dram_tensor("consts", (P, 3 * ot_n + vaug_w), F32,
                            kind="ExternalInput").ap()
    outd = nc.dram_tensor("out", (s, d), F32, kind="ExternalOutput").ap()

    cw = 1024   # q/k/w DMA chunk width (bf16 -> 2KB per-partition lines)
    cwv = 512   # v chunk width (finer so the first v-group starts sooner)

    with tile.TileContext(nc) as tc:
        with tc.tile_pool(name="singles", bufs=1) as singles:
            # warm-up scratch first: memset on the (idle) Vector queue so
            # the PE warm-up matmuls can fire as early as possible
            wu = singles.tile([P, 512], BF16)
            nc.vector.memset(wu, 0.0)
            cst = singles.tile([P, 3 * ot_n + vaug_w], F32)
            nc.scalar.dma_start(out=cst, in_=consts)
            bq_t = cst[:, 0:ot_n]
            bk_t = cst[:, ot_n:2 * ot_n]
            mb_t = cst[:, 2 * ot_n:3 * ot_n]
            bv_t = cst[:, 3 * ot_n:3 * ot_n + vaug_w]
            ident = singles.tile([P, P], F32)
            idr = singles.tile([P, P], BF16)

            with tc.tile_pool(name="qp", bufs=3) as qpp, \
                 tc.tile_pool(name="kp", bufs=3) as kpp, \
                 tc.tile_pool(name="vaug", bufs=st_n) as vaugp, \
                 tc.tile_pool(name="xq", bufs=kt_n * (s // cw)) as xqp, \
                 tc.tile_pool(name="wq", bufs=kt_n * (d // cw)) as wqp, \
                 tc.tile_pool(name="xk", bufs=kt_n * (s // cw)) as xkp, \
                 tc.tile_pool(name="wk", bufs=kt_n * (d // cw)) as wkp:

                def load_chunks(pool, dram, tag, w=cw):
                    """chunks[kt][c] = [P, w] bf16 slice of dram."""
                    tiles = [[pool.tile([P, w], BF16, tag=tag,
                                        name=f"{tag}_{kt}_{c}")
                              for c in range(dram.shape[1] // w)]
                             for kt in range(kt_n)]
                    return tiles

                def issue_chunk_dmas(eng, tiles, dram, w=cw):
                    """Issue one tensor's chunk DMAs kt-major on `eng`'s
                    queue (queues run in parallel per engine)."""
                    for c in range(len(tiles[0])):
                        for kt in range(kt_n):
                            eng.dma_start(
                                out=tiles[kt][c],
                                in_=dram[kt * P:(kt + 1) * P,
                                         c * w:(c + 1) * w],
                            )

                def wslice(tiles, kt, col0, width, w=cw):
                    c, off = divmod(col0, w)
                    assert off + width <= w
                    return tiles[kt][c][:, off:off + width]

                # ============ Phase V: v-projection into v_aug ============
                vaug_tiles = []
                with tc.tile_pool(name="xv", bufs=kt_n * (s // cwv)) as xvp, \
                     tc.tile_pool(name="wv", bufs=kt_n * (d // cwv)) as wvp, \
                     tc.tile_pool(name="vpsum", bufs=4, space="PSUM") as vpsum:
                    xv_t = load_chunks(xvp, vT, "xv", w=cwv)
                    wv_t = load_chunks(wvp, wvT, "wv", w=cwv)
                    # x-chunks on the Sync DMA queue, w-chunks on the
                    # (otherwise idle) GpSimd queue - parallel streams
                    issue_chunk_dmas(nc.sync, xv_t, vT, w=cwv)
                    issue_chunk_dmas(nc.gpsimd, wv_t, wvT, w=cwv)
                    # queue the main-loop operand DMAs right behind
                    xq_t = load_chunks(xqp, qT, "xq")
                    wq_t = load_chunks(wqp, wqT, "wq")
                    xk_t = load_chunks(xkp, kT, "xk")
                    wk_t = load_chunks(wkp, wkT, "wk")
                    issue_chunk_dmas(nc.sync, xq_t, qT)
                    issue_chunk_dmas(nc.gpsimd, wq_t, wqT)
                    issue_chunk_dmas(nc.sync, xk_t, kT)
                    issue_chunk_dmas(nc.gpsimd, wk_t, wkT)

                    # PE clock warm-up: the HAM un-throttles (1.2 -> 2.4
                    # GHz) only after ~3.4us of sustained PE activity, and
                    # the first real matmuls are DMA-gated anyway. Burn
                    # matmuls on a zeroed scratch tile so the clock is
                    # warm (and the pipeline hot) when operands land.
                    for _ in range(12):
                        wps = vpsum.tile([P, ch], F32, tag="vpsum")
                        nc.tensor.matmul(
                            wps, wu[:, 0:P], wu[:, 0:ch],
                            start=True, stop=True,
                        )
                    # identity (for the output transposes, needed much
                    # later) built after the DMA issues
                    make_identity(nc, ident)
                    nc.vector.tensor_copy(idr, ident)

                    bv_g = bv_t.rearrange("p (g c) -> p g c", c=dh + 1)
                    oc_n = d // ch
                    va_gs = []
                    for st in range(st_n):
                        va = vaugp.tile([P, vaug_w], BF16, tag="vaug",
                                        name=f"vaug_{st}")
                        vaug_tiles.append(va)
                        va_gs.append(va.rearrange("p (g c) -> p g c",
                                                  c=dh + 1))
                    # oc-outer matches the DMA arrival order (all chunk-0
                    # columns land before chunk-1) so the PE is never
                    # waiting on a late column while early ones sit unused
                    for oc in range(oc_n):
                        for st in range(st_n):
                            if oc == 0 and st < 4:
                                # the early v-phase is DMA-paced; keep the
                                # PE active so the HAM clock stays warm
                                for _ in range(2):
                                    wps = vpsum.tile([P, ch], F32,
                                                     tag="vpsum")
                                    nc.tensor.matmul(
                                        wps, wu[:, 0:P], wu[:, 0:ch],
                                        start=True, stop=True,
                                    )
                            ps = vpsum.tile([P, ch], F32, tag="vpsum")
                            for kt in range(kt_n):
                                nc.tensor.matmul(
                                    ps,
                                    wslice(xv_t, kt, st * P, P, w=cwv),
                                    wslice(wv_t, kt, oc * ch, ch, w=cwv),
                                    start=(kt == 0),
                                    stop=(kt == kt_n - 1),
                                )
                            g0 = oc * (ch // dh)
                            gn = ch // dh
                            nc.vector.tensor_tensor(
                                out=va_gs[st][:, g0:g0 + gn, 0:dh],
                                in0=ps.rearrange("p (g c) -> p g c", c=dh),
                                in1=bv_g[:, g0:g0 + gn, 0:dh],
                                op=mybir.AluOpType.add,
                            )
                    for st in range(st_n):
                        nc.vector.tensor_copy(
                            va_gs[st][:, :, dh:dh + 1],
                            bv_g[:, :, dh:dh + 1]
                        )

                # ============ Main software-pipelined loop ============
                with tc.tile_pool(name="exp", bufs=18) as expp, \
                     tc.tile_pool(name="ots", bufs=2) as otsp, \
                     tc.tile_pool(name="rcp", bufs=8) as rcpp, \
                     tc.tile_pool(name="fin", bufs=8) as finp, \
                     tc.tile_pool(name="ppsum", bufs=1, space="PSUM") as ppsum, \
                     tc.tile_pool(name="spsum", bufs=2, space="PSUM") as spsum, \
                     tc.tile_pool(name="opsum", bufs=1, space="PSUM") as opsum, \
                     tc.tile_pool(name="tpsum", bufs=1, space="PSUM") as tpsum:

                    qp_tiles = {}
                    kp_tiles = {}

                    # ---- projection emitter (q,k for one pair) ----
                    # each yield emits ONE matmul; bias-add drains are
                    # emitted inline after each 8-matmul group
                    def make_proj_gen(p):
                        def gen():
                            for name, w_t, x_t, b_t, tp, store in (
                                ("qp", wq_t, xq_t, bq_t, qpp, qp_tiles),
                                ("kp", wk_t, xk_t, bk_t, kpp, kp_tiles),
                            ):
                                po = tp.tile([P, s], BF16, tag=name,
                                             name=f"{name}_{p}")
                                store[p] = po
                                for sc in range(ch_n):
                                    ps = ppsum.tile([P, ch], F32, tag="ppsum")
                                    for kt in range(kt_n):
                                        nc.tensor.matmul(
                                            ps,
                                            wslice(w_t, kt, p * P, P),
                                            wslice(x_t, kt, sc * ch, ch),
                                            start=(kt == 0),
                                            stop=(kt == kt_n - 1),
                                        )
                                        yield
                                    nc.vector.tensor_scalar_add(
                                        po[:, sc * ch:(sc + 1) * ch],
                                        ps,
                                        b_t[:, p:p + 1],
                                    )
                        return gen()

                    def run_gen(g, n):
                        if g is None:
                            return
                        for _ in range(n):
                            try:
                                next(g)
                            except StopIteration:
                                return

                    # ---- AV emitter for one group ----
                    def make_av_gen(p, qc, exp_tiles, ot_ps):
                        def gen():
                            for kt in range(st_n):
                                for hp in range(hp_n):
                                    hh = p * hp_n + hp
                                    nc.tensor.matmul(
                                        ot_ps[:, hp * ch:(hp + 1) * ch],
                                        vaug_tiles[kt][
                                            :, hh * (dh + 1):(hh + 1) * (dh + 1)
                                        ],
                                        exp_tiles[kt][:, hp * ch:(hp + 1) * ch],
                                        start=(kt == 0),
                                        stop=(kt == st_n - 1),
                                        skip_group_check=True,
                                    )
                                    yield
                        return gen()

                    # ---- drain emitter (transpose+normalize+store) ----
                    def tr_tpsum():
                        t = tpsum.tile([P, dh + 2], F32, tag="tpsum",
                                       name="tr_t")
                        return t

                    def tr_ppsum():
                        # reuse ppsum's tag/arena (idle in the epilogue)
                        t = ppsum.tile([P, ch], F32, tag="ppsum",
                                       name="tr_p")
                        return t[:, 0:dh + 2]

                    def make_drain_gen(p, qc, ots, tr_allocs=None):
                        tr_allocs = tr_allocs or [tr_tpsum]

                        def gen():
                            ti = 0
                            for qb in range(qb_n):
                                fin = finp.tile([P, P], F32, tag="fin",
                                                name=f"fin_{p}_{qc}_{qb}")
                                for hp in range(hp_n):
                                    tr = tr_allocs[ti % len(tr_allocs)]()
                                    ti += 1
                                    nc.tensor.matmul(
                                        tr,
                                        ots[:, hp * ch + qb * P:
                                            hp * ch + (qb + 1) * P],
                                        idr[0:dh + 1, 0:dh + 2],
                                        start=True,
                                        stop=True,
                                    )
                                    yield
                                    rcp = rcpp.tile([P, 1], F32, tag="rcp")
                                    nc.vector.reciprocal(rcp, tr[:, dh:dh + 1])
                                    nc.vector.tensor_scalar_mul(
                                        fin[:, hp * dh:(hp + 1) * dh],
                                        tr[:, 0:dh],
                                        rcp,
                                    )
                                row0 = qc * ch + qb * P
                                nc.sync.dma_start(
                                    out=outd[row0:row0 + P, p * P:(p + 1) * P],
                                    in_=fin,
                                )
                        return gen()

                    # ---- prologue: project pair 0 fully ----
                    run_gen(make_proj_gen(0), 10 ** 6)

                    def chain_proj():
                        for p2 in range(1, ot_n):
                            yield from make_proj_gen(p2)
                    proj_gen = chain_proj()

                    groups = [(p, qc) for p in range(ot_n)
                              for qc in range(ch_n)]
                    av_gen = None      # AV of previous group
                    drain_gen = None   # drain of group before that
                    pend = []          # [(p, qc, exp_tiles)] awaiting AV

                    for gi, (p, qc) in enumerate(groups):
                        qp_t, kp_t = qp_tiles[p], kp_tiles[p]
                        exp_tiles = []
                        for kt in range(st_n):
                            # filler first (older groups); scores pair
                            # next; the tiny transpose right after the
                            # pair absorbs its drain window
                            run_gen(av_gen, 2 if kt < 4 else 3)
                            run_gen(proj_gen, 2 if av_gen is not None else 3)
                            sc_ps = spsum.tile([P, 2 * ch], F32, tag="spsum")
                            for hp in range(hp_n):
                                r0 = hp * dh
                                nc.tensor.matmul(
                                    sc_ps[:, hp * ch:(hp + 1) * ch],
                                    kp_t[r0:r0 + dh, kt * P:(kt + 1) * P],
                                    qp_t[r0:r0 + dh, qc * ch:(qc + 1) * ch],
                                    start=True,
                                    stop=True,
                                )
                            et = expp.tile([P, 2 * ch], BF16, tag="exp",
                                           name=f"exp_{p}_{qc}_{kt}")
                            nc.scalar.activation(
                                et,
                                sc_ps,
                                mybir.ActivationFunctionType.Exp,
                                bias=mb_t[:, kt:kt + 1],
                            )
                            exp_tiles.append(et)
                            run_gen(drain_gen, 1)
                        # finish any leftover filler from this group
                        run_gen(av_gen, 10 ** 6)
                        run_gen(drain_gen, 10 ** 6)
                        # previous group's AV psum -> bf16 SBUF, schedule
                        # its drain for the next group's step loop
                        if pend:
                            pp, pqc, pot = pend.pop(0)
                            ots = otsp.tile([dh + 1, 2 * ch], BF16, tag="ots")
                            nc.vector.tensor_copy(ots[:, 0:ch], pot[:, 0:ch])
                            nc.vector.tensor_copy(ots[:, ch:2 * ch],
                                                  pot[:, ch:2 * ch])
                            drain_gen = make_drain_gen(pp, pqc, ots)
                        ot_ps = opsum.tile([dh + 1, 2 * ch], F32, tag="opsum")
                        av_gen = make_av_gen(p, qc, exp_tiles, ot_ps)
                        pend.append((p, qc, ot_ps))

                    # ---- epilogue: interleave the last group's AV with
                    # the second-to-last group's drain, then drain the
                    # last group with double-buffered transpose psum
                    # (tpsum + the now-idle ppsum alternate) ----
                    for _ in range(8):
                        run_gen(av_gen, 2)
                        run_gen(drain_gen, 1)
                    run_gen(av_gen, 10 ** 6)
                    run_gen(drain_gen, 10 ** 6)
                    while pend:
                        pp, pqc, pot = pend.pop(0)
                        ots = otsp.tile([dh + 1, 2 * ch], BF16, tag="ots")
                        nc.vector.tensor_copy(ots[:, 0:ch], pot[:, 0:ch])
                        nc.vector.tensor_copy(ots[:, ch:2 * ch],
                                              pot[:, ch:2 * ch])
                        run_gen(make_drain_gen(pp, pqc, ots,
                                               tr_allocs=[tr_tpsum,
                                                          tr_ppsum]),
                                10 ** 6)

    return nc


_CACHE = {}


def _get_compiled(masked=False):
    key = "nc"
    if key not in _CACHE:
        nc = build_bass()
        nc.compile()
        _CACHE[key] = nc
    return _CACHE[key]


def kernel(q, k, v, mask, Wq, bq, Wk, bk, Wv, bv):
    q = np.asarray(q, dtype=np.float32)
    k = np.asarray(k, dtype=np.float32)
    v = np.asarray(v, dtype=np.float32)
    mask = np.asarray(mask, dtype=np.float32)
    Wq = np.asarray(Wq, dtype=np.float32)
    Wk = np.asarray(Wk, dtype=np.float32)
    Wv = np.asarray(Wv, dtype=np.float32)
    bq = np.asarray(bq, dtype=np.float32)
    bk = np.asarray(bk, dtype=np.float32)
    bv = np.asarray(bv, dtype=np.float32)

    nc = _get_compiled()

    ot_n = D // P
    st_n = S // P
    scale = 1.0 / float(np.sqrt(DH))
    # fold the softmax scale into Wq/bq (exact fp32 rescale pre-cast)
    wqT = np.ascontiguousarray(Wq.T * scale).astype(BF16NP)
    wkT = np.ascontiguousarray(Wk.T).astype(BF16NP)
    wvT = np.ascontiguousarray(Wv.T).astype(BF16NP)
    bqT = (bq * scale).reshape(ot_n, P).T
    bkT = bk.reshape(ot_n, P).T
    bv_aug = np.concatenate(
        [bv.reshape(H, DH), np.ones((H, 1), np.float32)], axis=1
    ).reshape(-1).astype(np.float32)
    bvB = np.broadcast_to(bv_aug, (P, H * (DH + 1)))

    in_maps = []
    for b in range(B):
        mbias = (-10000.0 * (1.0 - mask[b])).astype(np.float32)
        consts = np.ascontiguousarray(np.concatenate(
            [bqT, bkT, mbias.reshape(st_n, P).T, bvB], axis=1
        ).astype(np.float32))
        in_maps.append({
            "qT": np.ascontiguousarray(q[b].T).astype(BF16NP),
            "kT": np.ascontiguousarray(k[b].T).astype(BF16NP),
            "vT": np.ascontiguousarray(v[b].T).astype(BF16NP),
            "wqT": wqT,
            "wkT": wkT,
            "wvT": wvT,
            "consts": consts,
        })

    _CACHE["in_maps"] = in_maps
    res = run_bass_kernel_spmd(nc, in_maps, core_ids=list(range(N_CORES)))
    out = np.stack([res.results[b]["out"] for b in range(B)], axis=0)
    return out.astype(np.float32)
